# revision 29
# baseline (speedup 1.0000x reference)
"""Trainium2 Bass kernel for nn_CPDTail (CP-decomposed depthwise conv along H).

Computation:
    out[b,c,h,w] = sum_{k<3} sum_{r<8} x[b,h+k,c,r,w] * weight[c,r,k] + bias[c]
with x: (16,58,128,8,56) f32, weight: (128,8,3) f32, bias: (128,) f32,
out: (16,128,56,56) f32.

Sharding: data-parallel over batch B across the 8 NeuronCores (2 per core).

Per-core strategy (default): the per-channel (r,k) contraction runs on the
TensorEngine as block-diagonal matmuls. Channels are processed in 4 groups of
32 with the rank dim split in halves; SBUF partitions hold p=(c_sub*4+r_low)
so the K=128 contraction covers (32 channels x 4 ranks). The stationary
operand lhsT[p, m<32] is block-diagonal (built host-side from `weight`); the
2 rank halves x 3 kernel taps accumulate in PSUM via h-shifted rhs slices
(tile_position pins each group's 32-aligned output strip).

The kernel's PE math runs in bf16 either way, so the host prep rounds x to
bf16 and uploads it in a DMA-optimal layout (BL, C, Hp, G, NH, W): every
8-row block DMA is one contiguous 7KB run per partition straight into the
per-batch SBUF tile (no staging, no on-chip cast). HBM traffic is 13.3 MB
x-read + 1.6 MB bf16 out-write per core (the out store is bf16; the host
upcasts to fp32), against ~358 GB/s/core HBM: the stream (~39 us) is the
roofline and the PE chunk span (~2.4 us warm) matches the ~2.6 us block
cadence (ridge). PSUM evacuation (+bias, bf16 cast) rides ScalarE
(Identity activation with AP bias) so no engine FIFO couples the stream to
matmul completion; stores ride the ScalarE DGE ring, the x stream owns the
SyncE ring. Measured ~54-56 us/core (vs 97-111 us for the fp32-upload v4
baseline); l2 relative error ~2.8e-3 (bf16 rounding) vs the 2e-2 gate.

Fallbacks (KERNEL_VARIANT env): v1 = exact-fp32 VectorE MAC, v3 = fp32 PE
(exact), v4 = prior fp32-upload default, v5 = fp32-upload + contiguous
stream, v6/v9/v13/v15 = bf16-upload family (default v13 = best measured:
2-row first chunk, 2-row final block, 2 stores/batch), v18 = 2-ranks-fp8
experiment (correct at l2 1.37e-2 but slower: the fp8->bf16 upcast chain
de-pipelines the stream).
"""

import os
import sys

if "/opt/trn_rl_repo" not in sys.path:
    sys.path.insert(0, "/opt/trn_rl_repo")

import ml_dtypes
import numpy as np

import concourse.bass as bass
import concourse.tile as tile
from concourse import bacc, mybir
from concourse import bass_utils

# Problem shape (hardcoded; kernel.py must be self-contained).
B, Hp, C, R, W = 16, 58, 128, 8, 56
KS = 3
H = Hp - 2  # 56 output rows (PAD=1, STRIDE=1)
NCORES = 8
BL = B // NCORES  # batches per core

G = 4          # channel groups
CG = C // G    # channels per group = 32
HC = 8         # output h rows per chunk
NCHUNK = H // HC  # 7 chunks

F32 = mybir.dt.float32
BF16 = mybir.dt.bfloat16
AL = mybir.AluOpType
ACT_COPY = mybir.ActivationFunctionType.Copy

VARIANT = os.environ.get("KERNEL_VARIANT", "v13")
STAGE_BUFS = int(os.environ.get("STAGE_BUFS", "4"))
STREAM_RINGS = int(os.environ.get("STREAM_RINGS", "1"))
OUT_COALESCE = int(os.environ.get("OUT_COALESCE", "1"))
# chunk-index groups per output store (rows 0-19, 20-43, 44-55)
_OGRP4 = {0: [0, 1, 2], 1: [0, 1, 2], 2: [0, 1, 2],
          3: [3, 4, 5], 4: [3, 4, 5], 5: [3, 4, 5],
          6: [6, 7], 7: [6, 7]}
_ob_cache = [None]


def _new_nc():
    return bacc.Bacc("TRN2", target_bir_lowering=False, debug=False,
                     num_devices=NCORES)


# ---------------------------------------------------------------- V1 (fp32)
def _build_v1():
    """fp32 vector-engine kernel: 24 per-partition-scalar MAC terms."""
    nc = _new_nc()
    x_d = nc.dram_tensor("x", (BL, Hp, C, R, W), F32, kind="ExternalInput").ap()
    wb_d = nc.dram_tensor("wb", (C, R * KS + 1), F32, kind="ExternalInput").ap()
    o_d = nc.dram_tensor("out", (BL, C, H, W), F32, kind="ExternalOutput").ap()

    with tile.TileContext(nc) as tc:
        with (
            tc.tile_pool(name="consts", bufs=1) as consts,
            tc.tile_pool(name="xp", bufs=1) as xp,
            tc.tile_pool(name="accp", bufs=2) as accp,
        ):
            wb_sb = consts.tile([C, R * KS + 1], F32)
            nc.sync.dma_start(wb_sb[:], wb_d[:])

            for b in range(BL):
                # SBUF layout [c; hp, r, w] keeps DRAM-contiguous (r,w) runs.
                x_sb = xp.tile([C, Hp, R, W], F32, name=f"x_{b}", tag="x")
                nc.sync.dma_start(x_sb[:], x_d[b].rearrange("hp c r w -> c hp r w"))

                acc = accp.tile([C, H, W], F32, name=f"acc_{b}", tag="acc")
                first = True
                for r in range(R):
                    for k in range(KS):
                        xin = x_sb[:, k:k + H, r, :]
                        wsc = wb_sb[:, r * KS + k:r * KS + k + 1]
                        if first:
                            # acc = x*w + bias  (single-src, 2x mode)
                            nc.vector.tensor_scalar(
                                acc[:], xin, wsc, wb_sb[:, R * KS:],
                                AL.mult, AL.add)
                            first = False
                        else:
                            # acc = x*w + acc  (fused MAC)
                            nc.vector.scalar_tensor_tensor(
                                acc[:], xin, wsc, acc[:], AL.mult, AL.add)
                nc.sync.dma_start(o_d[b], acc[:])
    nc.compile()
    return nc


def _prep_v1(x, w, bias):
    wb = np.ascontiguousarray(
        np.concatenate([w.reshape(C, R * KS), bias.reshape(C, 1)], axis=1))
    return [{"x": x[c * BL:(c + 1) * BL], "wb": wb} for c in range(NCORES)]


# ------------------------------------------------- V2 (PE bf16 block-diagonal)
# hp staging blocks: chunk j needs hp rows [8j, 8j+10); block j ends at 8j+10.
_BLOCKS = [(0, 10)] + [(10 + 8 * i, 8) for i in range(6)]
NH = 2          # rank halves
RH = R // NH    # ranks per half = 4


def _build_v2():
    nc = _new_nc()
    x_d = nc.dram_tensor("x2", (BL, C, G, NH, Hp, W), F32,
                         kind="ExternalInput").ap()
    w_d = nc.dram_tensor("lhsT", (C, G, NH, KS, CG), BF16,
                         kind="ExternalInput").ap()
    b_d = nc.dram_tensor("bias", (C, 1), F32, kind="ExternalInput").ap()
    o_d = nc.dram_tensor("out", (BL, C, H, W), F32, kind="ExternalOutput").ap()

    with tile.TileContext(nc) as tc:
        with (
            tc.tile_pool(name="consts", bufs=1) as consts,
            tc.tile_pool(name="stage", bufs=3) as stage,
            tc.tile_pool(name="xbp", bufs=2) as xbp,
            tc.tile_pool(name="psum", bufs=4, space="PSUM") as psump,
            tc.tile_pool(name="outp", bufs=3) as outp,
        ):
            lhsT_sb = consts.tile([C, G, NH, KS, CG], BF16)
            nc.sync.dma_start(lhsT_sb[:], w_d[:])
            bias_sb = consts.tile([C, 1], F32)
            nc.sync.dma_start(bias_sb[:], b_d[:])

            for b in range(BL):
                # bf16 copy of this batch's x, partition p=(c_sub*4+r_low).
                xb = xbp.tile([C, G, NH, Hp, W], BF16, name=f"xb_{b}", tag="xb")
                for j, (r0, nr) in enumerate(_BLOCKS):
                    xs = stage.tile([C, G, NH, 10, W], F32, name=f"xs_{b}_{j}",
                                    tag="xs")
                    nc.sync.dma_start(xs[:, :, :, :nr, :],
                                      x_d[b, :, :, :, r0:r0 + nr, :])
                    # fp32 -> bf16 cast: split groups across ScalarE/VectorE.
                    nc.scalar.activation(xb[:, 0:2, :, r0:r0 + nr, :],
                                         xs[:, 0:2, :, :nr, :], ACT_COPY)
                    nc.vector.tensor_copy(xb[:, 2:4, :, r0:r0 + nr, :],
                                          xs[:, 2:4, :, :nr, :])

                    # chunk j is ready once block j is cast
                    h0 = HC * j
                    ps = psump.tile([C, HC * W], F32, name=f"ps_{b}_{j}",
                                    tag="ps")
                    for g in range(G):
                        nmm = 0
                        for hf in range(NH):
                            for k in range(KS):
                                nc.tensor.matmul(
                                    ps[CG * g:CG * (g + 1), :],
                                    lhsT_sb[:, g, hf, k, :],
                                    xb[:, g, hf, h0 + k:h0 + k + HC, :],
                                    start=(nmm == 0),
                                    stop=(nmm == NH * KS - 1),
                                    tile_position=(0, CG * g))
                                nmm += 1
                    ob = outp.tile([C, HC, W], F32, name=f"ob_{b}_{j}",
                                   tag="ob")
                    nc.vector.tensor_scalar(
                        ob[:].rearrange("c h w -> c (h w)"), ps[:],
                        bias_sb[:, 0:1], None, AL.add)
                    nc.sync.dma_start(o_d[b, :, h0:h0 + HC, :], ob[:])
    nc.compile()
    return nc


def _prep_v2(x, w, bias):
    # x2[b, cs*4+rh, g, hf, hp, w] = x[b, hp, 32g+cs, 4hf+rh, w]
    x2 = np.ascontiguousarray(
        x.reshape(B, Hp, G, CG, NH, RH, W).transpose(0, 3, 5, 2, 4, 1, 6)
        .reshape(B, C, G, NH, Hp, W))
    # lhsT[cs*4+rh, g, hf, k, m] = w[32g+m, 4hf+rh, k] if cs == m else 0
    wt = w.reshape(G, CG, NH, RH, KS)  # (g, cs, hf, rh, k)
    arr = np.zeros((CG, RH, G, NH, KS, CG), np.float32)
    for cs in range(CG):
        arr[cs, :, :, :, :, cs] = wt[:, cs, :, :, :].transpose(2, 0, 1, 3)
    lhsT = np.ascontiguousarray(
        arr.reshape(C, G, NH, KS, CG).astype(ml_dtypes.bfloat16))
    bias2 = np.ascontiguousarray(bias.reshape(C, 1))
    return [{"x2": x2[c * BL:(c + 1) * BL], "lhsT": lhsT, "bias": bias2}
            for c in range(NCORES)]


def _build_v3():
    """V2 structure but fp32 matmuls (rate experiment / exact path)."""
    nc = _new_nc()
    x_d = nc.dram_tensor("x2", (BL, C, G, NH, Hp, W), F32,
                         kind="ExternalInput").ap()
    w_d = nc.dram_tensor("lhsT", (C, G, NH, KS, CG), F32,
                         kind="ExternalInput").ap()
    b_d = nc.dram_tensor("bias", (C, 1), F32, kind="ExternalInput").ap()
    o_d = nc.dram_tensor("out", (BL, C, H, W), F32, kind="ExternalOutput").ap()

    with tile.TileContext(nc) as tc:
        with (
            tc.tile_pool(name="consts", bufs=1) as consts,
            tc.tile_pool(name="xbp", bufs=1) as xbp,
            tc.tile_pool(name="psum", bufs=4, space="PSUM") as psump,
            tc.tile_pool(name="outp", bufs=3) as outp,
        ):
            lhsT_sb = consts.tile([C, G, NH, KS, CG], F32)
            nc.sync.dma_start(lhsT_sb[:], w_d[:])
            bias_sb = consts.tile([C, 1], F32)
            nc.sync.dma_start(bias_sb[:], b_d[:])

            for b in range(BL):
                xb = xbp.tile([C, G, NH, Hp, W], F32, name=f"xb_{b}", tag="xb")
                for j, (r0, nr) in enumerate(_BLOCKS):
                    nc.sync.dma_start(xb[:, :, :, r0:r0 + nr, :],
                                      x_d[b, :, :, :, r0:r0 + nr, :])
                    h0 = HC * j
                    ps = psump.tile([C, HC * W], F32, name=f"ps_{b}_{j}",
                                    tag="ps")
                    for g in range(G):
                        nmm = 0
                        for hf in range(NH):
                            for k in range(KS):
                                nc.tensor.matmul(
                                    ps[CG * g:CG * (g + 1), :],
                                    lhsT_sb[:, g, hf, k, :],
                                    xb[:, g, hf, h0 + k:h0 + k + HC, :],
                                    start=(nmm == 0),
                                    stop=(nmm == NH * KS - 1),
                                    tile_position=(0, CG * g))
                                nmm += 1
                    ob = outp.tile([C, HC, W], F32, name=f"ob_{b}_{j}",
                                   tag="ob")
                    nc.vector.tensor_scalar(
                        ob[:].rearrange("c h w -> c (h w)"), ps[:],
                        bias_sb[:, 0:1], None, AL.add)
                    nc.sync.dma_start(o_d[b, :, h0:h0 + HC, :], ob[:])
    nc.compile()
    return nc


def _prep_v3(x, w, bias):
    x2 = np.ascontiguousarray(
        x.reshape(B, Hp, G, CG, NH, RH, W).transpose(0, 3, 5, 2, 4, 1, 6)
        .reshape(B, C, G, NH, Hp, W))
    wt = w.reshape(G, CG, NH, RH, KS)
    arr = np.zeros((CG, RH, G, NH, KS, CG), np.float32)
    for cs in range(CG):
        arr[cs, :, :, :, :, cs] = wt[:, cs, :, :, :].transpose(2, 0, 1, 3)
    lhsT = np.ascontiguousarray(arr.reshape(C, G, NH, KS, CG))
    bias2 = np.ascontiguousarray(bias.reshape(C, 1))
    return [{"x2": x2[c * BL:(c + 1) * BL], "lhsT": lhsT, "bias": bias2}
            for c in range(NCORES)]


# ---------------------------------------------------- V4 (tuned V2 pipeline)
# Variable-size output chunks so the PE can start after only 6 hp rows are
# resident, plus a small tail chunk. Chunk j consumes hp rows [h0, h0+hc+2).
_CHUNKS4 = [(0, 4)] + [(4 + 8 * i, 8) for i in range(6)] + [(52, 4)]
# Staging block j delivers exactly the extra hp rows chunk j needs.
_BLOCKS4 = [(0, 6), (6, 8), (14, 8), (22, 8), (30, 8), (38, 8), (46, 8),
            (54, 4)]


def _build_v4():
    nc = _new_nc()
    x_d = nc.dram_tensor("x2", (BL, C, G, NH, Hp, W), F32,
                         kind="ExternalInput").ap()
    w_d = nc.dram_tensor("lhsT", (C, G, NH, KS, CG), BF16,
                         kind="ExternalInput").ap()
    b_d = nc.dram_tensor("bias", (C, 1), F32, kind="ExternalInput").ap()
    o_d = nc.dram_tensor("out", (BL, C, H, W), F32, kind="ExternalOutput").ap()

    with tile.TileContext(nc) as tc:
        with (
            tc.tile_pool(name="consts", bufs=1) as consts,
            tc.tile_pool(name="stage", bufs=STAGE_BUFS) as stage,
            tc.tile_pool(name="xbp", bufs=2) as xbp,
            tc.tile_pool(name="psum", bufs=4, space="PSUM") as psump,
            tc.tile_pool(name="outp", bufs=3) as outp,
        ):
            # x stream owns the SP ring; constants ride the ACT HW-DGE ring
            # and are issued after the first x block so the stream leads.
            lhsT_sb = consts.tile([C, G, NH, KS, CG], BF16)
            bias_sb = consts.tile([C, 1], F32)
            consts_loaded = False

            for b in range(BL):
                xb = xbp.tile([C, G, NH, Hp, W], BF16, name=f"xb_{b}", tag="xb")
                for j, ((h0, hc), (r0, nr)) in enumerate(zip(_CHUNKS4,
                                                             _BLOCKS4)):
                    xs = stage.tile([C, G, NH, 8, W], F32, name=f"xs_{b}_{j}",
                                    tag="xs")
                    ring = (nc.scalar if (STREAM_RINGS == 2 and (b * 8 + j) % 2)
                            else nc.sync)
                    ring.dma_start(xs[:, :, :, :nr, :],
                                   x_d[b, :, :, :, r0:r0 + nr, :])
                    if not consts_loaded:
                        nc.scalar.dma_start(lhsT_sb[:], w_d[:])
                        nc.scalar.dma_start(bias_sb[:], b_d[:])
                        consts_loaded = True
                    # fp32 -> bf16 cast: ~1/4 on ScalarE, 3/4 on VectorE.
                    nc.scalar.activation(xb[:, 0:1, :, r0:r0 + nr, :],
                                         xs[:, 0:1, :, :nr, :], ACT_COPY)
                    nc.vector.tensor_copy(xb[:, 1:4, :, r0:r0 + nr, :],
                                          xs[:, 1:4, :, :nr, :])

                    n = hc * W
                    ps = psump.tile([C, HC * W], F32, name=f"ps_{b}_{j}",
                                    tag="ps")
                    for g in range(G):
                        nmm = 0
                        for hf in range(NH):
                            for k in range(KS):
                                nc.tensor.matmul(
                                    ps[CG * g:CG * (g + 1), :n],
                                    lhsT_sb[:, g, hf, k, :],
                                    xb[:, g, hf, h0 + k:h0 + k + hc, :],
                                    start=(nmm == 0),
                                    stop=(nmm == NH * KS - 1),
                                    tile_position=(0, CG * g))
                                nmm += 1
                    if OUT_COALESCE:
                        grp = _OGRP4[j]
                        if grp[0] == j:
                            ob = outp.tile([C, 24, W], F32,
                                           name=f"ob_{b}_{grp[0]}", tag="ob")
                            _ob_cache[0] = ob
                        ob = _ob_cache[0]
                        off = h0 - _CHUNKS4[grp[0]][0]
                        nc.vector.tensor_scalar(
                            ob[:, off:off + hc, :]
                            .rearrange("c h w -> c (h w)"), ps[:, :n],
                            bias_sb[:, 0:1], None, AL.add)
                        if grp[-1] == j:
                            g0 = _CHUNKS4[grp[0]][0]
                            rows = h0 + hc - g0
                            nc.scalar.dma_start(o_d[b, :, g0:g0 + rows, :],
                                                ob[:, :rows, :])
                    else:
                        ob = outp.tile([C, HC, W], F32, name=f"ob_{b}_{j}",
                                       tag="ob")
                        nc.vector.tensor_scalar(
                            ob[:].rearrange("c h w -> c (h w)")[:, :n],
                            ps[:, :n], bias_sb[:, 0:1], None, AL.add)
                        nc.scalar.dma_start(o_d[b, :, h0:h0 + hc, :],
                                            ob[:, :hc, :])
    nc.compile()
    return nc


# ------------------------------------------------- V5 (contiguous stream + bf16 out)
# DRAM x layout (BL, C, Hp, G, NH, W): partition p = cs*4+rh holds rows hp
# contiguously (448 elem/row), so an nr-row block DMA is ONE contiguous
# 14336B run per partition (vs 8 x 1792B in v4) -> bigger descriptors.
# SBUF xb uses the same free-dim order, so the fp32->bf16 cast is a flat
# contiguous copy done entirely on VectorE; PSUM evacuation (+bias, cast to
# bf16) moves to ScalarE so no engine FIFO couples the cast stream to matmul
# completion (the v4 bottleneck: DVE's evac waited on MMs, delaying the next
# cast, which gated staging-buffer reuse and thus the DMA descriptor feed).
# Output is stored bf16 (halves store traffic); the host upcasts to fp32.


def _build_v5():
    nc = _new_nc()
    x_d = nc.dram_tensor("x5", (BL, C, Hp, G, NH, W), F32,
                         kind="ExternalInput").ap()
    w_d = nc.dram_tensor("lhsT", (C, G, NH, KS, CG), BF16,
                         kind="ExternalInput").ap()
    b_d = nc.dram_tensor("bias", (C, 1), F32, kind="ExternalInput").ap()
    o_d = nc.dram_tensor("out", (BL, C, H, W), BF16, kind="ExternalOutput").ap()

    with tile.TileContext(nc) as tc:
        with (
            tc.tile_pool(name="consts", bufs=1) as consts,
            tc.tile_pool(name="stage", bufs=STAGE_BUFS) as stage,
            tc.tile_pool(name="xbp", bufs=2) as xbp,
            tc.tile_pool(name="psum", bufs=4, space="PSUM") as psump,
            tc.tile_pool(name="outp", bufs=3) as outp,
        ):
            lhsT_sb = consts.tile([C, G, NH, KS, CG], BF16)
            bias_sb = consts.tile([C, 1], F32)
            consts_loaded = False

            for b in range(BL):
                xb = xbp.tile([C, Hp, G, NH, W], BF16, name=f"xb_{b}", tag="xb")
                for j, ((h0, hc), (r0, nr)) in enumerate(zip(_CHUNKS4,
                                                             _BLOCKS4)):
                    xs = stage.tile([C, 8, G, NH, W], F32, name=f"xs_{b}_{j}",
                                    tag="xs")
                    nc.sync.dma_start(xs[:, :nr], x_d[b, :, r0:r0 + nr])
                    if not consts_loaded:
                        nc.scalar.dma_start(lhsT_sb[:], w_d[:])
                        nc.scalar.dma_start(bias_sb[:], b_d[:])
                        consts_loaded = True
                    # fp32 -> bf16 cast: contiguous copy, all on VectorE
                    nc.vector.tensor_copy(xb[:, r0:r0 + nr], xs[:, :nr])

                    n = hc * W
                    ps = psump.tile([C, HC * W], F32, name=f"ps_{b}_{j}",
                                    tag="ps")
                    for g in range(G):
                        nmm = 0
                        for hf in range(NH):
                            for k in range(KS):
                                nc.tensor.matmul(
                                    ps[CG * g:CG * (g + 1), :n],
                                    lhsT_sb[:, g, hf, k, :],
                                    xb[:, h0 + k:h0 + k + hc, g, hf, :],
                                    start=(nmm == 0),
                                    stop=(nmm == NH * KS - 1),
                                    tile_position=(0, CG * g))
                                nmm += 1
                    grp = _OGRP4[j]
                    if grp[0] == j:
                        ob = outp.tile([C, 24, W], BF16,
                                       name=f"ob_{b}_{grp[0]}", tag="ob")
                        _ob_cache[0] = ob
                    ob = _ob_cache[0]
                    off = h0 - _CHUNKS4[grp[0]][0]
                    # PSUM evac + bias + bf16 cast on ScalarE (Identity allows
                    # an AP bias; its PWL table is exact for a linear function)
                    nc.scalar.activation(
                        ob[:, off:off + hc, :].rearrange("c h w -> c (h w)"),
                        ps[:, :n], mybir.ActivationFunctionType.Identity,
                        bias=bias_sb[:, 0:1])
                    if grp[-1] == j:
                        g0 = _CHUNKS4[grp[0]][0]
                        rows = h0 + hc - g0
                        nc.scalar.dma_start(o_d[b, :, g0:g0 + rows, :],
                                            ob[:, :rows, :])
    nc.compile()
    return nc


def _prep_v5(x, w, bias):
    # x5[b, cs*4+rh, hp, g, hf, w] = x[b, hp, 32g+cs, 4hf+rh, w]
    x5 = np.ascontiguousarray(
        x.reshape(B, Hp, G, CG, NH, RH, W).transpose(0, 3, 5, 1, 2, 4, 6)
        .reshape(B, C, Hp, G, NH, W))
    wt = w.reshape(G, CG, NH, RH, KS)  # (g, cs, hf, rh, k)
    arr = np.zeros((CG, RH, G, NH, KS, CG), np.float32)
    for cs in range(CG):
        arr[cs, :, :, :, :, cs] = wt[:, cs, :, :, :].transpose(2, 0, 1, 3)
    lhsT = np.ascontiguousarray(
        arr.reshape(C, G, NH, KS, CG).astype(ml_dtypes.bfloat16))
    bias2 = np.ascontiguousarray(bias.reshape(C, 1))
    return [{"x5": x5[c * BL:(c + 1) * BL], "lhsT": lhsT, "bias": bias2}
            for c in range(NCORES)]


# ------------------------------------------------- V6 (bf16 x upload)
# The kernel rounds x to bf16 before the PE anyway, so do the cast host-side
# and upload x as bf16: HBM read traffic halves (26.6 -> 13.3 MB/core) with
# identical accuracy. No staging, no on-chip cast: blocks DMA straight into
# the per-batch xb tile. PE chunk span (~2.5us warm) ~ matches the ~2.6us
# block arrival, so the kernel sits right at the ridge.


def _build_v6():
    nc = _new_nc()
    x_d = nc.dram_tensor("x6", (BL, C, Hp, G, NH, W), BF16,
                         kind="ExternalInput").ap()
    w_d = nc.dram_tensor("lhsT", (C, G, NH, KS, CG), BF16,
                         kind="ExternalInput").ap()
    b_d = nc.dram_tensor("bias", (C, 1), F32, kind="ExternalInput").ap()
    o_d = nc.dram_tensor("out", (BL, C, H, W), BF16, kind="ExternalOutput").ap()

    with tile.TileContext(nc) as tc:
        with (
            tc.tile_pool(name="consts", bufs=1) as consts,
            tc.tile_pool(name="xbp", bufs=2) as xbp,
            tc.tile_pool(name="psum", bufs=4, space="PSUM") as psump,
            tc.tile_pool(name="outp", bufs=3) as outp,
        ):
            lhsT_sb = consts.tile([C, G, NH, KS, CG], BF16)
            bias_sb = consts.tile([C, 1], F32)
            consts_loaded = False

            for b in range(BL):
                xb = xbp.tile([C, Hp, G, NH, W], BF16, name=f"xb_{b}", tag="xb")
                for j, ((h0, hc), (r0, nr)) in enumerate(zip(_CHUNKS4,
                                                             _BLOCKS4)):
                    nc.sync.dma_start(xb[:, r0:r0 + nr], x_d[b, :, r0:r0 + nr])
                    if not consts_loaded:
                        nc.scalar.dma_start(lhsT_sb[:], w_d[:])
                        nc.scalar.dma_start(bias_sb[:], b_d[:])
                        consts_loaded = True

                    n = hc * W
                    ps = psump.tile([C, HC * W], F32, name=f"ps_{b}_{j}",
                                    tag="ps")
                    for g in range(G):
                        nmm = 0
                        for hf in range(NH):
                            for k in range(KS):
                                nc.tensor.matmul(
                                    ps[CG * g:CG * (g + 1), :n],
                                    lhsT_sb[:, g, hf, k, :],
                                    xb[:, h0 + k:h0 + k + hc, g, hf, :],
                                    start=(nmm == 0),
                                    stop=(nmm == NH * KS - 1),
                                    tile_position=(0, CG * g))
                                nmm += 1
                    grp = _OGRP4[j]
                    if grp[0] == j:
                        ob = outp.tile([C, 24, W], BF16,
                                       name=f"ob_{b}_{grp[0]}", tag="ob")
                        _ob_cache[0] = ob
                    ob = _ob_cache[0]
                    off = h0 - _CHUNKS4[grp[0]][0]
                    nc.scalar.activation(
                        ob[:, off:off + hc, :].rearrange("c h w -> c (h w)"),
                        ps[:, :n], mybir.ActivationFunctionType.Identity,
                        bias=bias_sb[:, 0:1])
                    if grp[-1] == j:
                        g0 = _CHUNKS4[grp[0]][0]
                        rows = h0 + hc - g0
                        nc.scalar.dma_start(o_d[b, :, g0:g0 + rows, :],
                                            ob[:, :rows, :])
    nc.compile()
    return nc


def _prep_v6(x, w, bias):
    x6 = np.ascontiguousarray(
        x.reshape(B, Hp, G, CG, NH, RH, W).transpose(0, 3, 5, 1, 2, 4, 6)
        .reshape(B, C, Hp, G, NH, W).astype(ml_dtypes.bfloat16))
    wt = w.reshape(G, CG, NH, RH, KS)
    arr = np.zeros((CG, RH, G, NH, KS, CG), np.float32)
    for cs in range(CG):
        arr[cs, :, :, :, :, cs] = wt[:, cs, :, :, :].transpose(2, 0, 1, 3)
    lhsT = np.ascontiguousarray(
        arr.reshape(C, G, NH, KS, CG).astype(ml_dtypes.bfloat16))
    bias2 = np.ascontiguousarray(bias.reshape(C, 1))
    return [{"x6": x6[c * BL:(c + 1) * BL], "lhsT": lhsT, "bias": bias2}
            for c in range(NCORES)]


# ------------------------------------------------- V9 (V6 + fewer stores)
# Same pipeline as V6 but output coalesced into 2 stores per batch (rows
# 0-43 and 44-55): 20 HWDGE DMAs total instead of 24, easing the 8-sem-lane
# recycling pressure that occasionally bubbles the x-stream descriptor feed.
_OGRP9 = {0: [0, 1, 2, 3, 4, 5], 1: [0, 1, 2, 3, 4, 5], 2: [0, 1, 2, 3, 4, 5],
          3: [0, 1, 2, 3, 4, 5], 4: [0, 1, 2, 3, 4, 5], 5: [0, 1, 2, 3, 4, 5],
          6: [6, 7], 7: [6, 7]}


def _build_v9(chunks=_CHUNKS4, blocks=_BLOCKS4):
    nc = _new_nc()
    x_d = nc.dram_tensor("x6", (BL, C, Hp, G, NH, W), BF16,
                         kind="ExternalInput").ap()
    w_d = nc.dram_tensor("lhsT", (C, G, NH, KS, CG), BF16,
                         kind="ExternalInput").ap()
    b_d = nc.dram_tensor("bias", (C, 1), F32, kind="ExternalInput").ap()
    o_d = nc.dram_tensor("out", (BL, C, H, W), BF16, kind="ExternalOutput").ap()

    with tile.TileContext(nc) as tc:
        with (
            tc.tile_pool(name="consts", bufs=1) as consts,
            tc.tile_pool(name="xbp", bufs=2) as xbp,
            tc.tile_pool(name="psum", bufs=4, space="PSUM") as psump,
            tc.tile_pool(name="outp", bufs=2) as outp,
        ):
            lhsT_sb = consts.tile([C, G, NH, KS, CG], BF16)
            bias_sb = consts.tile([C, 1], F32)
            consts_loaded = False

            for b in range(BL):
                xb = xbp.tile([C, Hp, G, NH, W], BF16, name=f"xb_{b}", tag="xb")
                for j, ((h0, hc), (r0, nr)) in enumerate(zip(chunks, blocks)):
                    nc.sync.dma_start(xb[:, r0:r0 + nr], x_d[b, :, r0:r0 + nr])
                    if not consts_loaded:
                        nc.scalar.dma_start(lhsT_sb[:], w_d[:])
                        nc.scalar.dma_start(bias_sb[:], b_d[:])
                        consts_loaded = True

                    n = hc * W
                    ps = psump.tile([C, HC * W], F32, name=f"ps_{b}_{j}",
                                    tag="ps")
                    for g in range(G):
                        nmm = 0
                        for hf in range(NH):
                            for k in range(KS):
                                nc.tensor.matmul(
                                    ps[CG * g:CG * (g + 1), :n],
                                    lhsT_sb[:, g, hf, k, :],
                                    xb[:, h0 + k:h0 + k + hc, g, hf, :],
                                    start=(nmm == 0),
                                    stop=(nmm == NH * KS - 1),
                                    tile_position=(0, CG * g))
                                nmm += 1
                    grp = _OGRP9[j]
                    if grp[0] == j:
                        ob = outp.tile([C, 44, W], BF16,
                                       name=f"ob_{b}_{grp[0]}", tag="ob")
                        _ob_cache[0] = ob
                    ob = _ob_cache[0]
                    off = h0 - chunks[grp[0]][0]
                    nc.scalar.activation(
                        ob[:, off:off + hc, :].rearrange("c h w -> c (h w)"),
                        ps[:, :n], mybir.ActivationFunctionType.Identity,
                        bias=bias_sb[:, 0:1])
                    if grp[-1] == j:
                        g0 = chunks[grp[0]][0]
                        rows = h0 + hc - g0
                        nc.scalar.dma_start(o_d[b, :, g0:g0 + rows, :],
                                            ob[:, :rows, :])
    nc.compile()
    return nc


# ------------------------------------------- V10 (V9 + 16-row DMA blocks)
_BLOCKS10 = [(0, 6), (6, 16), (22, 16), (38, 16), (54, 4)]


def _build_v10(psum_bufs=4, blocks=_BLOCKS10):
    nc = _new_nc()
    x_d = nc.dram_tensor("x6", (BL, C, Hp, G, NH, W), BF16,
                         kind="ExternalInput").ap()
    w_d = nc.dram_tensor("lhsT", (C, G, NH, KS, CG), BF16,
                         kind="ExternalInput").ap()
    b_d = nc.dram_tensor("bias", (C, 1), F32, kind="ExternalInput").ap()
    o_d = nc.dram_tensor("out", (BL, C, H, W), BF16, kind="ExternalOutput").ap()

    with tile.TileContext(nc) as tc:
        with (
            tc.tile_pool(name="consts", bufs=1) as consts,
            tc.tile_pool(name="xbp", bufs=2) as xbp,
            tc.tile_pool(name="psum", bufs=psum_bufs, space="PSUM") as psump,
            tc.tile_pool(name="outp", bufs=2) as outp,
        ):
            lhsT_sb = consts.tile([C, G, NH, KS, CG], BF16)
            bias_sb = consts.tile([C, 1], F32)
            consts_loaded = False

            for b in range(BL):
                xb = xbp.tile([C, Hp, G, NH, W], BF16, name=f"xb_{b}", tag="xb")
                for r0, nr in blocks:
                    nc.sync.dma_start(xb[:, r0:r0 + nr], x_d[b, :, r0:r0 + nr])
                    if not consts_loaded:
                        nc.scalar.dma_start(lhsT_sb[:], w_d[:])
                        nc.scalar.dma_start(bias_sb[:], b_d[:])
                        consts_loaded = True
                for j, (h0, hc) in enumerate(_CHUNKS4):
                    n = hc * W
                    ps = psump.tile([C, HC * W], F32, name=f"ps_{b}_{j}",
                                    tag="ps")
                    for g in range(G):
                        nmm = 0
                        for hf in range(NH):
                            for k in range(KS):
                                nc.tensor.matmul(
                                    ps[CG * g:CG * (g + 1), :n],
                                    lhsT_sb[:, g, hf, k, :],
                                    xb[:, h0 + k:h0 + k + hc, g, hf, :],
                                    start=(nmm == 0),
                                    stop=(nmm == NH * KS - 1),
                                    tile_position=(0, CG * g))
                                nmm += 1
                    grp = _OGRP9[j]
                    if grp[0] == j:
                        ob = outp.tile([C, 44, W], BF16,
                                       name=f"ob_{b}_{grp[0]}", tag="ob")
                        _ob_cache[0] = ob
                    ob = _ob_cache[0]
                    off = h0 - _CHUNKS4[grp[0]][0]
                    nc.scalar.activation(
                        ob[:, off:off + hc, :].rearrange("c h w -> c (h w)"),
                        ps[:, :n], mybir.ActivationFunctionType.Identity,
                        bias=bias_sb[:, 0:1])
                    if grp[-1] == j:
                        g0 = _CHUNKS4[grp[0]][0]
                        rows = h0 + hc - g0
                        nc.scalar.dma_start(o_d[b, :, g0:g0 + rows, :],
                                            ob[:, :rows, :])
    nc.compile()
    return nc


# ------------------------------------------------- V18 (2 ranks in fp8)
# Partition map p = rh*32 + cs (rank-major) so rank pairs occupy contiguous
# partition halves. x ships as three tensors: X0 bf16 = ranks 0-3 (hf0 slots,
# all 128 partitions), X1 bf16 = ranks 4-5 (hf1 slots, partitions 0-63), XC
# fp8e4m3 = ranks 6-7 (staged, upcast to bf16 by the otherwise-idle VectorE
# into hf1 slots of partitions 64-127). Weights stay bf16, so the exact
# output error is 1.36e-2 (vs the 2e-2 gate; measured host-side in numpy).
# HBM x-read drops 13.3 -> 10.8 MB/core; the matmul structure is unchanged.
F8E4 = mybir.dt.float8e4
_XCSPLIT = 36  # fp8/X1 part boundary (no v13 block spans row 36)


def _build_v18():
    nc = _new_nc()
    x0_d = nc.dram_tensor("x0", (BL, C, Hp, G, W), BF16,
                          kind="ExternalInput").ap()
    x1_d = nc.dram_tensor("x1", (BL, 64, Hp, G, W), BF16,
                          kind="ExternalInput").ap()
    xc_d = nc.dram_tensor("xc", (BL, 64, Hp, G, W), F8E4,
                          kind="ExternalInput").ap()
    w_d = nc.dram_tensor("lhsT", (C, G, NH, KS, CG), BF16,
                         kind="ExternalInput").ap()
    b_d = nc.dram_tensor("bias", (C, 1), F32, kind="ExternalInput").ap()
    o_d = nc.dram_tensor("out", (BL, C, H, W), BF16, kind="ExternalOutput").ap()

    parts = [(0, Hp)]  # single consolidated hf1/fp8 transfer per batch

    with tile.TileContext(nc) as tc:
        with (
            tc.tile_pool(name="consts", bufs=1) as consts,
            tc.tile_pool(name="xbp", bufs=2) as xbp,
            tc.tile_pool(name="xcsp", bufs=2) as xcsp,
            tc.tile_pool(name="psum", bufs=4, space="PSUM") as psump,
            tc.tile_pool(name="outp", bufs=2) as outp,
        ):
            lhsT_sb = consts.tile([C, G, NH, KS, CG], BF16)
            bias_sb = consts.tile([C, 1], F32)
            consts_loaded = False

            for b in range(BL):
                xb = xbp.tile([C, NH, Hp, G, W], BF16, name=f"xb_{b}",
                              tag="xb")
                # fp8 rank pair staged per part; upcast slices follow blocks
                xcs = [xcsp.tile([C, pr, G, W], F8E4, name=f"xcs_{b}_{i}",
                                 tag=f"xcs{i}")
                       for i, (p0, pr) in enumerate(parts)]
                # issue order: X0 block 0 first (fast chunk-0 start), then
                # the hf1 bf16/fp8 parts, then the remaining X0 blocks
                for j, (r0, nr) in enumerate(_BLOCKS13):
                    nc.sync.dma_start(xb[:, 0, r0:r0 + nr],
                                      x0_d[b, :, r0:r0 + nr])
                    if j < len(parts):
                        p0, pr = parts[j]
                        nc.sync.dma_start(xb[0:64, 1, p0:p0 + pr],
                                          x1_d[b, :, p0:p0 + pr])
                        nc.scalar.dma_start(xcs[j][64:128, :],
                                            xc_d[b, :, p0:p0 + pr])
                        if not consts_loaded:
                            nc.scalar.dma_start(lhsT_sb[:], w_d[:])
                            nc.scalar.dma_start(bias_sb[:], b_d[:])
                            consts_loaded = True
                    # fp8 -> bf16 upcast for this block's rows (VectorE)
                    pi = min(len(parts) - 1,
                             0 if r0 < _XCSPLIT else 1)
                    off = r0 - parts[pi][0]
                    nc.vector.tensor_copy(xb[64:128, 1, r0:r0 + nr],
                                          xcs[pi][64:128, off:off + nr])

                for j, (h0, hc) in enumerate(_CHUNKS13):
                    n = hc * W
                    ps = psump.tile([C, HC * W], F32, name=f"ps_{b}_{j}",
                                    tag="ps")
                    for g in range(G):
                        nmm = 0
                        for hf in range(NH):
                            for k in range(KS):
                                nc.tensor.matmul(
                                    ps[CG * g:CG * (g + 1), :n],
                                    lhsT_sb[:, g, hf, k, :],
                                    xb[:, hf, h0 + k:h0 + k + hc, g, :],
                                    start=(nmm == 0),
                                    stop=(nmm == NH * KS - 1),
                                    tile_position=(0, CG * g))
                                nmm += 1
                    grp = _OGRP13[j]
                    if grp[0] == j:
                        ob = outp.tile([C, 44, W], BF16,
                                       name=f"ob_{b}_{grp[0]}", tag="ob")
                        _ob_cache[0] = ob
                    ob = _ob_cache[0]
                    off = h0 - _CHUNKS13[grp[0]][0]
                    nc.scalar.activation(
                        ob[:, off:off + hc, :].rearrange("c h w -> c (h w)"),
                        ps[:, :n], mybir.ActivationFunctionType.Identity,
                        bias=bias_sb[:, 0:1])
                    if grp[-1] == j:
                        g0 = _CHUNKS13[grp[0]][0]
                        rows = h0 + hc - g0
                        nc.scalar.dma_start(o_d[b, :, g0:g0 + rows, :],
                                            ob[:, :rows, :])
    nc.compile()
    return nc


def _prep_v18(x, w, bias):
    # p = rh*32 + cs: rank-major partition map
    xt = np.ascontiguousarray(
        x.reshape(B, Hp, G, CG, R, W).transpose(0, 4, 3, 1, 2, 5))
    # xt: (B, R, CG, Hp, G, W)
    x0 = np.ascontiguousarray(
        xt[:, 0:4].reshape(B, C, Hp, G, W).astype(ml_dtypes.bfloat16))
    x1 = np.ascontiguousarray(
        xt[:, 4:6].reshape(B, 64, Hp, G, W).astype(ml_dtypes.bfloat16))
    xc = np.ascontiguousarray(
        xt[:, 6:8].reshape(B, 64, Hp, G, W).astype(ml_dtypes.float8_e4m3))
    # lhsT[rh*32+cs, g, hf, k, m] = delta(cs==m) * w[32g+m, 4hf+rh, k]
    wt = w.reshape(G, CG, NH, 4, KS)  # (g, cs, hf, rh, k)
    arr = np.zeros((4, CG, G, NH, KS, CG), np.float32)
    for cs in range(CG):
        arr[:, cs, :, :, :, cs] = wt[:, cs, :, :, :].transpose(2, 0, 1, 3)
    lhsT = np.ascontiguousarray(
        arr.reshape(C, G, NH, KS, CG).astype(ml_dtypes.bfloat16))
    bias2 = np.ascontiguousarray(bias.reshape(C, 1))
    return [{"x0": x0[c * BL:(c + 1) * BL], "x1": x1[c * BL:(c + 1) * BL],
             "xc": xc[c * BL:(c + 1) * BL], "lhsT": lhsT, "bias": bias2}
            for c in range(NCORES)]


# ---------------------------------------- V13 (fill/tail-trimmed schedule)
# First chunk 2 rows (starts after a 4-row block: ~1.2us fill), last chunk 6
# rows fed by a final 2-row block (less transfer+receipt exposed at the
# tail). Store groups: rows 0-41 and 42-55 per batch.
_CHUNKS13 = [(0, 2), (2, 8), (10, 8), (18, 8), (26, 8), (34, 8), (42, 8),
             (50, 6)]
_BLOCKS13 = [(0, 4), (4, 8), (12, 8), (20, 8), (28, 8), (36, 8), (44, 8),
             (52, 4), (56, 2)]
_OGRP13 = {0: [0, 1, 2, 3, 4, 5], 1: [0, 1, 2, 3, 4, 5],
           2: [0, 1, 2, 3, 4, 5], 3: [0, 1, 2, 3, 4, 5],
           4: [0, 1, 2, 3, 4, 5], 5: [0, 1, 2, 3, 4, 5],
           6: [6, 7], 7: [6, 7]}


def _build_v13(chunks=_CHUNKS13, blocks=_BLOCKS13, ogrp=_OGRP13):
    nc = _new_nc()
    x_d = nc.dram_tensor("x6", (BL, C, Hp, G, NH, W), BF16,
                         kind="ExternalInput").ap()
    w_d = nc.dram_tensor("lhsT", (C, G, NH, KS, CG), BF16,
                         kind="ExternalInput").ap()
    b_d = nc.dram_tensor("bias", (C, 1), F32, kind="ExternalInput").ap()
    o_d = nc.dram_tensor("out", (BL, C, H, W), BF16, kind="ExternalOutput").ap()

    with tile.TileContext(nc) as tc:
        with (
            tc.tile_pool(name="consts", bufs=1) as consts,
            tc.tile_pool(name="xbp", bufs=2) as xbp,
            tc.tile_pool(name="psum", bufs=4, space="PSUM") as psump,
            tc.tile_pool(name="outp", bufs=2) as outp,
        ):
            lhsT_sb = consts.tile([C, G, NH, KS, CG], BF16)
            bias_sb = consts.tile([C, 1], F32)
            consts_loaded = False

            for b in range(BL):
                xb = xbp.tile([C, Hp, G, NH, W], BF16, name=f"xb_{b}", tag="xb")
                for r0, nr in blocks:
                    nc.sync.dma_start(xb[:, r0:r0 + nr], x_d[b, :, r0:r0 + nr])
                    if not consts_loaded:
                        nc.scalar.dma_start(lhsT_sb[:], w_d[:])
                        nc.scalar.dma_start(bias_sb[:], b_d[:])
                        consts_loaded = True
                for j, (h0, hc) in enumerate(chunks):
                    n = hc * W
                    ps = psump.tile([C, HC * W], F32, name=f"ps_{b}_{j}",
                                    tag="ps")
                    for g in range(G):
                        nmm = 0
                        for hf in range(NH):
                            for k in range(KS):
                                nc.tensor.matmul(
                                    ps[CG * g:CG * (g + 1), :n],
                                    lhsT_sb[:, g, hf, k, :],
                                    xb[:, h0 + k:h0 + k + hc, g, hf, :],
                                    start=(nmm == 0),
                                    stop=(nmm == NH * KS - 1),
                                    tile_position=(0, CG * g))
                                nmm += 1
                    grp = ogrp[j]
                    if grp[0] == j:
                        ob = outp.tile([C, 44, W], BF16,
                                       name=f"ob_{b}_{grp[0]}", tag="ob")
                        _ob_cache[0] = ob
                    ob = _ob_cache[0]
                    off = h0 - chunks[grp[0]][0]
                    nc.scalar.activation(
                        ob[:, off:off + hc, :].rearrange("c h w -> c (h w)"),
                        ps[:, :n], mybir.ActivationFunctionType.Identity,
                        bias=bias_sb[:, 0:1])
                    if grp[-1] == j:
                        g0 = chunks[grp[0]][0]
                        rows = h0 + hc - g0
                        nc.scalar.dma_start(o_d[b, :, g0:g0 + rows, :],
                                            ob[:, :rows, :])
    nc.compile()
    return nc


# ------------------------------------------------- V8 (V6 + tail tuning)
# DMA blocks decoupled from compute chunks (tile auto-deps cover the MM ->
# block mapping): the final blocks shrink to 2 rows so less transfer+receipt
# latency sits exposed after the last chunk's data, and the last store is a
# lone 4-row chunk that issues as early as possible.
_BLOCKS8 = [(0, 6), (6, 8), (14, 8), (22, 8), (30, 8), (38, 8), (46, 8),
            (54, 2), (56, 2)]
_OGRP8 = {0: [0, 1, 2], 1: [0, 1, 2], 2: [0, 1, 2],
          3: [3, 4, 5], 4: [3, 4, 5], 5: [3, 4, 5], 6: [6], 7: [7]}


def _build_v8():
    nc = _new_nc()
    x_d = nc.dram_tensor("x6", (BL, C, Hp, G, NH, W), BF16,
                         kind="ExternalInput").ap()
    w_d = nc.dram_tensor("lhsT", (C, G, NH, KS, CG), BF16,
                         kind="ExternalInput").ap()
    b_d = nc.dram_tensor("bias", (C, 1), F32, kind="ExternalInput").ap()
    o_d = nc.dram_tensor("out", (BL, C, H, W), BF16, kind="ExternalOutput").ap()

    with tile.TileContext(nc) as tc:
        with (
            tc.tile_pool(name="consts", bufs=1) as consts,
            tc.tile_pool(name="xbp", bufs=2) as xbp,
            tc.tile_pool(name="psum", bufs=4, space="PSUM") as psump,
            tc.tile_pool(name="outp", bufs=3) as outp,
        ):
            lhsT_sb = consts.tile([C, G, NH, KS, CG], BF16)
            bias_sb = consts.tile([C, 1], F32)
            consts_loaded = False

            for b in range(BL):
                xb = xbp.tile([C, Hp, G, NH, W], BF16, name=f"xb_{b}", tag="xb")
                for bi, (r0, nr) in enumerate(_BLOCKS8):
                    nc.sync.dma_start(xb[:, r0:r0 + nr], x_d[b, :, r0:r0 + nr])
                    if not consts_loaded:
                        nc.scalar.dma_start(lhsT_sb[:], w_d[:])
                        nc.scalar.dma_start(bias_sb[:], b_d[:])
                        consts_loaded = True
                for j, (h0, hc) in enumerate(_CHUNKS4):
                    n = hc * W
                    ps = psump.tile([C, HC * W], F32, name=f"ps_{b}_{j}",
                                    tag="ps")
                    for g in range(G):
                        nmm = 0
                        for hf in range(NH):
                            for k in range(KS):
                                nc.tensor.matmul(
                                    ps[CG * g:CG * (g + 1), :n],
                                    lhsT_sb[:, g, hf, k, :],
                                    xb[:, h0 + k:h0 + k + hc, g, hf, :],
                                    start=(nmm == 0),
                                    stop=(nmm == NH * KS - 1),
                                    tile_position=(0, CG * g))
                                nmm += 1
                    grp = _OGRP8[j]
                    if grp[0] == j:
                        ob = outp.tile([C, 24, W], BF16,
                                       name=f"ob_{b}_{grp[0]}", tag="ob")
                        _ob_cache[0] = ob
                    ob = _ob_cache[0]
                    off = h0 - _CHUNKS4[grp[0]][0]
                    nc.scalar.activation(
                        ob[:, off:off + hc, :].rearrange("c h w -> c (h w)"),
                        ps[:, :n], mybir.ActivationFunctionType.Identity,
                        bias=bias_sb[:, 0:1])
                    if grp[-1] == j:
                        g0 = _CHUNKS4[grp[0]][0]
                        rows = h0 + hc - g0
                        nc.scalar.dma_start(o_d[b, :, g0:g0 + rows, :],
                                            ob[:, :rows, :])
    nc.compile()
    return nc


# ------------------------------------------------- V7 (V6 + PE/store tuning)
# MM issue order (hf,k)-outer, g-inner: adjacent matmuls target different
# col-groups so the four 32-wide group chains run concurrently in the PE
# array. PSUM bufs 6 for cross-chunk ILP; final store group split per chunk
# so the last store issues earlier.
_OGRP7 = {0: [0, 1, 2], 1: [0, 1, 2], 2: [0, 1, 2],
          3: [3, 4, 5], 4: [3, 4, 5], 5: [3, 4, 5], 6: [6], 7: [7]}


def _build_v7():
    nc = _new_nc()
    x_d = nc.dram_tensor("x6", (BL, C, Hp, G, NH, W), BF16,
                         kind="ExternalInput").ap()
    w_d = nc.dram_tensor("lhsT", (C, G, NH, KS, CG), BF16,
                         kind="ExternalInput").ap()
    b_d = nc.dram_tensor("bias", (C, 1), F32, kind="ExternalInput").ap()
    o_d = nc.dram_tensor("out", (BL, C, H, W), BF16, kind="ExternalOutput").ap()

    with tile.TileContext(nc) as tc:
        with (
            tc.tile_pool(name="consts", bufs=1) as consts,
            tc.tile_pool(name="xbp", bufs=2) as xbp,
            tc.tile_pool(name="psum", bufs=6, space="PSUM") as psump,
            tc.tile_pool(name="outp", bufs=3) as outp,
        ):
            lhsT_sb = consts.tile([C, G, NH, KS, CG], BF16)
            bias_sb = consts.tile([C, 1], F32)
            consts_loaded = False

            for b in range(BL):
                xb = xbp.tile([C, Hp, G, NH, W], BF16, name=f"xb_{b}", tag="xb")
                for j, ((h0, hc), (r0, nr)) in enumerate(zip(_CHUNKS4,
                                                             _BLOCKS4)):
                    nc.sync.dma_start(xb[:, r0:r0 + nr], x_d[b, :, r0:r0 + nr])
                    if not consts_loaded:
                        nc.scalar.dma_start(lhsT_sb[:], w_d[:])
                        nc.scalar.dma_start(bias_sb[:], b_d[:])
                        consts_loaded = True

                    n = hc * W
                    ps = psump.tile([C, HC * W], F32, name=f"ps_{b}_{j}",
                                    tag="ps")
                    for i, (hf, k) in enumerate((hf, k) for hf in range(NH)
                                                for k in range(KS)):
                        for g in range(G):
                            nc.tensor.matmul(
                                ps[CG * g:CG * (g + 1), :n],
                                lhsT_sb[:, g, hf, k, :],
                                xb[:, h0 + k:h0 + k + hc, g, hf, :],
                                start=(i == 0),
                                stop=(i == NH * KS - 1),
                                tile_position=(0, CG * g))
                    grp = _OGRP7[j]
                    if grp[0] == j:
                        ob = outp.tile([C, 24, W], BF16,
                                       name=f"ob_{b}_{grp[0]}", tag="ob")
                        _ob_cache[0] = ob
                    ob = _ob_cache[0]
                    off = h0 - _CHUNKS4[grp[0]][0]
                    nc.scalar.activation(
                        ob[:, off:off + hc, :].rearrange("c h w -> c (h w)"),
                        ps[:, :n], mybir.ActivationFunctionType.Identity,
                        bias=bias_sb[:, 0:1])
                    if grp[-1] == j:
                        g0 = _CHUNKS4[grp[0]][0]
                        rows = h0 + hc - g0
                        nc.scalar.dma_start(o_d[b, :, g0:g0 + rows, :],
                                            ob[:, :rows, :])
    nc.compile()
    return nc


_BUILDERS = {"v1": (_build_v1, _prep_v1), "v2": (_build_v2, _prep_v2),
             "v3": (_build_v3, _prep_v3), "v4": (_build_v4, _prep_v2),
             "v5": (_build_v5, _prep_v5), "v6": (_build_v6, _prep_v6),
             "v7": (_build_v7, _prep_v6), "v8": (_build_v8, _prep_v6),
             "v9": (_build_v9, _prep_v6),
             "v10": (_build_v10, _prep_v6),
             "v12": (lambda: _build_v10(psum_bufs=6, blocks=_BLOCKS4),
                     _prep_v6),
             "v13": (_build_v13, _prep_v6),
             "v18": (_build_v18, _prep_v18)}

# V15: v9 with the final DMA block shrunk to 2 rows (block 6 widened to 10)
# so only ~0.6us of transfer+receipt sits exposed behind the last chunk.
_BLOCKS15 = [(0, 6), (6, 8), (14, 8), (22, 8), (30, 8), (38, 8), (46, 10),
             (56, 2)]


def _build_v15():
    return _build_v9(chunks=_CHUNKS4, blocks=_BLOCKS15)


_BUILDERS["v15"] = (_build_v15, _prep_v6)
_NC_CACHE = {}


def _get_nc(variant):
    if variant not in _NC_CACHE:
        _NC_CACHE[variant] = _BUILDERS[variant][0]()
    return _NC_CACHE[variant]


def _run(inputs, trace=False, variant=None):
    variant = variant or VARIANT
    x = np.ascontiguousarray(np.asarray(inputs["x"], dtype=np.float32))
    w = np.ascontiguousarray(np.asarray(inputs["weight"], dtype=np.float32))
    bias = np.asarray(inputs["bias"], dtype=np.float32)
    assert x.shape == (B, Hp, C, R, W), x.shape

    nc = _get_nc(variant)
    in_maps = _BUILDERS[variant][1](x, w, bias)
    res = bass_utils.run_bass_kernel_spmd(
        nc, in_maps, core_ids=list(range(NCORES)), trace=trace)
    out = np.concatenate([r["out"] for r in res.results], axis=0)
    if out.dtype != np.float32:
        out = out.astype(np.float32)
    return out, res


def kernel(**inputs) -> np.ndarray:
    out, _ = _run(inputs, trace=False)
    return out

